# revision 6
# baseline (speedup 1.0000x reference)
"""Causal self-attention on 8 Trainium2 NeuronCores.

Problem: x[2,2048,2048] f32, W_qkv[2048,6144], W_out[2048,2048]
  qkv = x @ W_qkv; per-head causal softmax attention; out = attn @ W_out.

Sharding: core c handles batch b=c//4, head group hg=c%4 (4 of 16 heads).
Each core computes its heads' QKV projections, full causal attention for
those heads, and a partial output projection (its heads' rows of W_out).
Host sums the 4 partial outputs per batch.

Device kernel (per core, SPMD):
  Phase A: PE-transpose x -> xT slabs; qT/kT (per head) and v (all heads)
           via fp32r matmuls accumulating K=2048 in PSUM.
  Phase B: per head, per 512-wide query group: S^T = kT_blk.T @ qT (scores
           with keys on partitions), E = exp(scale*S^T) (ScalarE, fp32r out),
           causal mask on diagonal blocks via GpSimd affine_select,
           denom row via ones[128,1] matmul accumulation, attn_outT via
           v_blk as lhsT accumulation, then normalize by 1/denom broadcast
           (K=1 ones matmul) -- all without max-subtraction (scores ~N(0,1)).
  Phase C: y = sum_h attn_outT_h.T @ W_out_h rows, PSUM-accumulated over
           the 4 local heads.
"""
import math

import numpy as np

import concourse.bass as bass
import concourse.mybir as mybir
import concourse.tile as tile
from concourse import bacc
from concourse.bass_utils import run_bass_kernel_spmd
from concourse.masks import make_identity

B, T, D = 2, 2048, 2048
H, Hd = 16, 128
N_CORES = 8
HL = 4            # heads per core
DL = HL * Hd      # 512: local hidden slice
P = 128
KC = D // P       # 16 contraction chunks of 128
NTB = T // P      # 16 row blocks of 128
QTW = 512         # query-group width
NQT = T // QTW    # 4 query groups
SCALE = 1.0 / math.sqrt(Hd)

f32 = mybir.dt.float32
f32r = mybir.dt.float32r
AF = mybir.ActivationFunctionType


def build_program(reps: int = 1):
    nc = bacc.Bacc("TRN2", target_bir_lowering=False, debug=False,
                   num_devices=N_CORES)
    x = nc.dram_tensor("x", [T, D], f32r, kind="ExternalInput")
    wq = nc.dram_tensor("wq", [D, DL], f32r, kind="ExternalInput")
    wk = nc.dram_tensor("wk", [D, DL], f32r, kind="ExternalInput")
    wv = nc.dram_tensor("wv", [D, DL], f32r, kind="ExternalInput")
    wout = nc.dram_tensor("wout", [DL, D], f32r, kind="ExternalInput")
    y = nc.dram_tensor("y", [T, D], f32, kind="ExternalOutput")

    with tile.TileContext(nc) as tc:
        if reps > 1:
            with tc.For_i(0, reps, 1):
                _body(nc, tc, x, wq, wk, wv, wout, y)
        else:
            _body(nc, tc, x, wq, wk, wv, wout, y)
    nc.compile()
    return nc


def _body(nc, tc, x, wq, wk, wv, wout, y):
    with (
        tc.tile_pool(name="persist", bufs=1) as persist,
        tc.tile_pool(name="psum", bufs=8, space="PSUM") as psum,
    ):
        # f32r constants (built from f32 scratch, rounded via tensor_copy)
        ones_col = persist.tile([P, 1], f32r)      # lhsT for denom matmul
        ones_row = persist.tile([1, P], f32r)      # lhsT for K=1 broadcast
        ident = persist.tile([P, P], f32r)
        attnT_sb = persist.tile([P, HL, T], f32r)  # [Hd, h, Tq]
        with tc.tile_pool(name="init_scratch", bufs=1) as scratch:
            ones_f = scratch.tile([P, 1], f32)
            nc.vector.memset(ones_f[:], 1.0)
            nc.vector.tensor_copy(ones_col[:], ones_f[:])
            ones1_f = scratch.tile([1, P], f32)
            nc.vector.memset(ones1_f[:], 1.0)
            nc.vector.tensor_copy(ones_row[:], ones1_f[:])
            ident_f = scratch.tile([P, P], f32)
            make_identity(nc, ident_f[:])
            nc.vector.tensor_copy(ident[:], ident_f[:])

        with tc.tile_pool(name="qkv", bufs=1) as qkv_pool:
            qT_sb = qkv_pool.tile([P, HL, T], f32r)   # [Hd, h, Tq]
            kT_sb = qkv_pool.tile([P, HL, T], f32r)
            v_sb = qkv_pool.tile([P, NTB, DL], f32r)  # [Tk%128, kb, h*Hd]

            # ------------ Phase A: transpose + QKV projection ------------
            with (
                tc.tile_pool(name="a_stream", bufs=2) as astream,
                tc.tile_pool(name="a_xT", bufs=1) as xTpool,
            ):
                for s in range(NQT):  # 4 slabs of 512 T-rows
                    xT_sb = xTpool.tile([P, KC, QTW], f32r, tag="xT")
                    for tsub in range(QTW // P):
                        r0 = (s * 4 + tsub) * P
                        for half in range(2):
                            x_in = astream.tile([P, D // 2], f32r, tag="x_in")
                            nc.sync.dma_start(
                                x_in[:],
                                x.ap()[r0:r0 + P,
                                       half * (D // 2):(half + 1) * (D // 2)])
                            for dch in range(KC // 2):
                                dc = half * (KC // 2) + dch
                                tp = psum.tile([P, P], f32r, tag="ps",
                                               name="tp")
                                nc.tensor.transpose(
                                    tp[:], x_in[:, dch * P:(dch + 1) * P],
                                    ident[:])
                                nc.vector.tensor_copy(
                                    xT_sb[:, dc, tsub * P:(tsub + 1) * P],
                                    tp[:])
                    # qT / kT for the 4 local heads
                    for h in range(HL):
                        for wdram, dst in ((wq, qT_sb), (wk, kT_sb)):
                            wt = astream.tile([P, KC, Hd], f32r, tag="w_qk")
                            nc.sync.dma_start(
                                wt[:],
                                wdram.ap()[:, h * Hd:(h + 1) * Hd].rearrange(
                                    "(kc p) m -> p kc m", p=P))
                            ps = psum.tile([P, QTW], f32, tag="ps",
                                           name="qk_ps")
                            for kc in range(KC):
                                nc.tensor.matmul(
                                    ps[:], wt[:, kc, :], xT_sb[:, kc, :],
                                    start=(kc == 0), stop=(kc == KC - 1))
                            nc.vector.tensor_copy(
                                dst[:, h, s * QTW:(s + 1) * QTW], ps[:])
                    # v for all 4 heads (kc-outer so wv streams once per slab)
                    vps = [psum.tile([P, DL], f32, tag="ps", name=f"vps{i}")
                           for i in range(4)]
                    for kc in range(KC):
                        wvt = astream.tile([P, DL], f32r, tag="wv")
                        nc.sync.dma_start(
                            wvt[:], wv.ap()[kc * P:(kc + 1) * P, :])
                        for tsub in range(4):
                            nc.tensor.matmul(
                                vps[tsub][:],
                                xT_sb[:, kc, tsub * P:(tsub + 1) * P],
                                wvt[:],
                                start=(kc == 0), stop=(kc == KC - 1))
                    for tsub in range(4):
                        nc.vector.tensor_copy(
                            v_sb[:, s * 4 + tsub, :], vps[tsub][:])

            # ------------ Phase B: causal attention ----------------------
            with (
                tc.tile_pool(name="b_e", bufs=4) as epool,
                tc.tile_pool(name="b_small", bufs=2) as bsmall,
            ):
                for h in range(HL):
                    for qt in range(NQT):
                        nkb = (qt + 1) * 4
                        q_sl = slice(qt * QTW, (qt + 1) * QTW)
                        d_ps = psum.tile([1, QTW], f32, tag="ps", name="d_ps")
                        o_ps = psum.tile([P, QTW], f32, tag="ps", name="o_ps")
                        for kb in range(nkb):
                            s_ps = psum.tile([P, QTW], f32, tag="ps",
                                             name="s_ps")
                            nc.tensor.matmul(
                                s_ps[:],
                                kT_sb[:, h, kb * P:(kb + 1) * P],
                                qT_sb[:, h, q_sl],
                                start=True, stop=True)
                            e_sb = epool.tile([P, QTW], f32r, tag="e")
                            nc.scalar.activation(
                                e_sb[:], s_ps[:], AF.Exp, scale=float(SCALE))
                            if kb >= 4 * qt:
                                # diagonal block: keep where q >= k, i.e.
                                # j >= i + m*128 with m = kb - 4*qt
                                m = kb - 4 * qt
                                nc.gpsimd.affine_select(
                                    out=e_sb[:], in_=e_sb[:],
                                    compare_op=mybir.AluOpType.is_ge,
                                    fill=0.0,
                                    base=-m * P,
                                    channel_multiplier=-1,
                                    pattern=[[1, QTW]])
                            nc.tensor.matmul(
                                d_ps[:], ones_col[:], e_sb[:],
                                start=(kb == 0), stop=(kb == nkb - 1))
                            nc.tensor.matmul(
                                o_ps[:], v_sb[:, kb, h * Hd:(h + 1) * Hd],
                                e_sb[:],
                                start=(kb == 0), stop=(kb == nkb - 1))
                        den = bsmall.tile([1, QTW], f32, tag="den")
                        nc.scalar.copy(den[:], d_ps[:])
                        rec = bsmall.tile([1, QTW], f32r, tag="rec")
                        with nc.allow_low_precision(
                                reason="f32r reciprocal, 2^-19 rel"):
                            nc.vector.reciprocal(rec[:], den[:])
                        bc_ps = psum.tile([P, QTW], f32, tag="ps",
                                          name="bc_ps")
                        nc.tensor.matmul(
                            bc_ps[:], ones_row[:], rec[:],
                            start=True, stop=True)
                        bc_sb = bsmall.tile([P, QTW], f32, tag="bc")
                        nc.scalar.copy(bc_sb[:], bc_ps[:])
                        nc.vector.tensor_mul(
                            attnT_sb[:, h, q_sl], o_ps[:], bc_sb[:])

        # ------------ Phase C: output projection -------------------------
        with tc.tile_pool(name="c_pool", bufs=1) as cpool:
            wout_sb = cpool.tile([P, HL, D], f32r)
            nc.sync.dma_start(
                wout_sb[:], wout.ap().rearrange("(hl p) d -> p hl d", p=P))
            with tc.tile_pool(name="c_y", bufs=2) as ypool:
                for tb in range(NTB):
                    t_sl = slice(tb * P, (tb + 1) * P)
                    y_sb = ypool.tile([P, D], f32, tag="y")
                    for dc in range(D // QTW):
                        y_ps = psum.tile([P, QTW], f32, tag="ps", name="y_ps")
                        for h in range(HL):
                            nc.tensor.matmul(
                                y_ps[:],
                                attnT_sb[:, h, t_sl],
                                wout_sb[:, h, dc * QTW:(dc + 1) * QTW],
                                start=(h == 0), stop=(h == HL - 1))
                        nc.vector.tensor_copy(
                            y_sb[:, dc * QTW:(dc + 1) * QTW], y_ps[:])
                    nc.sync.dma_start(y.ap()[t_sl, :], y_sb[:])


def prepare_in_maps(x, W_qkv, W_out):
    x = np.ascontiguousarray(np.asarray(x), dtype=np.float32)
    W_qkv = np.ascontiguousarray(np.asarray(W_qkv), dtype=np.float32)
    W_out = np.ascontiguousarray(np.asarray(W_out), dtype=np.float32)
    Wr = W_qkv.reshape(D, 3, H, Hd)
    Wo = W_out.reshape(H, Hd, D)
    in_maps = []
    for c in range(N_CORES):
        b, hg = c // 4, c % 4
        hs = slice(hg * HL, (hg + 1) * HL)
        in_maps.append({
            "x": x[b],
            "wq": np.ascontiguousarray(Wr[:, 0, hs, :].reshape(D, DL)),
            "wk": np.ascontiguousarray(Wr[:, 1, hs, :].reshape(D, DL)),
            "wv": np.ascontiguousarray(Wr[:, 2, hs, :].reshape(D, DL)),
            "wout": np.ascontiguousarray(Wo[hs].reshape(DL, D)),
        })
    return in_maps


def combine_outputs(results):
    out = np.zeros((B, T, D), dtype=np.float32)
    for c in range(N_CORES):
        out[c // 4] += results[c]["y"]
    return out


_PROGRAM_CACHE = {}


def kernel(x, W_qkv, W_out):
    in_maps = prepare_in_maps(x, W_qkv, W_out)
    if 1 not in _PROGRAM_CACHE:
        _PROGRAM_CACHE[1] = build_program(1)
    nc = _PROGRAM_CACHE[1]
    res = run_bass_kernel_spmd(nc, in_maps, core_ids=list(range(N_CORES)))
    return combine_outputs(res.results)


# revision 7
# speedup vs baseline: 1.3776x; 1.3776x over previous
"""Causal self-attention on 8 Trainium2 NeuronCores.

Problem: x[2,2048,2048] f32, W_qkv[2048,6144], W_out[2048,2048]
  qkv = x @ W_qkv; per-head causal softmax attention; out = attn @ W_out.

Sharding: core c handles batch b=c//4, head group hg=c%4 (4 of 16 heads).
Each core computes its heads' QKV projections, full causal attention for
those heads, and a partial output projection (its heads' rows of W_out).
Host sums the 4 partial outputs per batch. x is shipped pre-transposed
(xT[b] = x[b].T) so the device needs no transposes: every matmul wants the
contraction dim (D or Tk or Hd) on partitions.

Device kernel (per core, SPMD, all matmuls fp32r):
  Phase A: qT/kT per head (lhsT=W chunk, rhs=xT slab) and v for all heads
           (lhsT=xT chunk, rhs=Wv block), K=2048 PSUM accumulation.
  Phase B: per head, per 512-wide query group: S^T = kT_blk.T @ qT (keys on
           partitions), E = exp(scale*S^T) (ScalarE -> fp32r), causal mask
           on diagonal blocks via GpSimd affine_select, denominator row via
           ones[128,1] matmul accumulation, attn_outT via v_blk-as-lhsT
           accumulation, normalize by 1/denom broadcast (K=1 ones matmul);
           no max-subtraction (scores ~N(0,1), exp is safe in fp32).
           Normalized attn_outT tiles stream to a DRAM scratch.
  Phase C: y = sum_h attn_outT_h.T @ W_out_h rows, PSUM-accumulated over
           the 4 local heads; attn_outT tiles reloaded from scratch.
"""
import math

import numpy as np

import concourse.bass as bass
import concourse.mybir as mybir
import concourse.tile as tile
from concourse import bacc
from concourse.bass_utils import run_bass_kernel_spmd

B, T, D = 2, 2048, 2048
H, Hd = 16, 128
N_CORES = 8
HL = 4            # heads per core
DL = HL * Hd      # 512: local hidden slice
P = 128
KC = D // P       # 16 contraction chunks of 128
NTB = T // P      # 16 row blocks of 128
QTW = 512         # query-group width
NQT = T // QTW    # 4 query groups
SCALE = 1.0 / math.sqrt(Hd)

f32 = mybir.dt.float32
f32r = mybir.dt.float32r
AF = mybir.ActivationFunctionType


def build_program(reps: int = 1):
    nc = bacc.Bacc("TRN2", target_bir_lowering=False, debug=False,
                   num_devices=N_CORES)
    xT = nc.dram_tensor("xT", [D, T], f32r, kind="ExternalInput")
    wq = nc.dram_tensor("wq", [D, DL], f32r, kind="ExternalInput")
    wk = nc.dram_tensor("wk", [D, DL], f32r, kind="ExternalInput")
    wv = nc.dram_tensor("wv", [D, DL], f32r, kind="ExternalInput")
    wout = nc.dram_tensor("wout", [DL, D], f32r, kind="ExternalInput")
    y = nc.dram_tensor("y", [T, D], f32, kind="ExternalOutput")

    with tile.TileContext(nc) as tc:
        if reps > 1:
            with tc.For_i(0, reps, 1):
                _body(nc, tc, xT, wq, wk, wv, wout, y)
        else:
            _body(nc, tc, xT, wq, wk, wv, wout, y)
    nc.compile()
    return nc


def _body(nc, tc, xT, wq, wk, wv, wout, y):
    with (
        tc.tile_pool(name="persist", bufs=1) as persist,
        tc.tile_pool(name="psum", bufs=8, space="PSUM") as psum,
        tc.tile_pool(name="dram", bufs=1, space="DRAM") as dram,
    ):
        # attn_outT scratch: [h, Hd, T] fp32r
        atT_dram = dram.tile([HL, Hd, T], f32r)

        # f32r constants (built from f32 scratch, rounded via tensor_copy)
        ones_col = persist.tile([P, 1], f32r)      # lhsT for denom matmul
        ones_row = persist.tile([1, P], f32r)      # lhsT for K=1 broadcast
        with tc.tile_pool(name="init_scratch", bufs=1) as scratch:
            ones_f = scratch.tile([P, 1], f32)
            nc.vector.memset(ones_f[:], 1.0)
            nc.vector.tensor_copy(ones_col[:], ones_f[:])
            ones1_f = scratch.tile([1, P], f32)
            nc.vector.memset(ones1_f[:], 1.0)
            nc.vector.tensor_copy(ones_row[:], ones1_f[:])

        with tc.tile_pool(name="qkv", bufs=1) as qkv_pool:
            qT_sb = qkv_pool.tile([P, HL, T], f32r)   # [Hd, h, Tq]
            kT_sb = qkv_pool.tile([P, HL, T], f32r)
            v_sb = qkv_pool.tile([P, NTB, DL], f32r)  # [Tk%128, kb, h*Hd]

            # ------------ Phase A: QKV projection ------------------------
            with (
                tc.tile_pool(name="a_xT", bufs=2) as xTpool,
                tc.tile_pool(name="a_w", bufs=2) as wpool,
                tc.tile_pool(name="a_wv", bufs=3) as wvpool,
            ):
                for s in range(NQT):  # 4 slabs of 512 T-cols
                    xTs = xTpool.tile([P, KC, QTW], f32r, tag="xT")
                    nc.sync.dma_start(
                        xTs[:],
                        xT.ap()[:, s * QTW:(s + 1) * QTW].rearrange(
                            "(kc p) t -> p kc t", p=P))
                    # qT / kT for the 4 local heads
                    for h in range(HL):
                        for wdram, dst in ((wq, qT_sb), (wk, kT_sb)):
                            wt = wpool.tile([P, KC, Hd], f32r, tag="w_qk")
                            nc.sync.dma_start(
                                wt[:],
                                wdram.ap()[:, h * Hd:(h + 1) * Hd].rearrange(
                                    "(kc p) m -> p kc m", p=P))
                            ps = psum.tile([P, QTW], f32, tag="ps",
                                           name="qk_ps")
                            for kc in range(KC):
                                nc.tensor.matmul(
                                    ps[:], wt[:, kc, :], xTs[:, kc, :],
                                    start=(kc == 0), stop=(kc == KC - 1))
                            nc.vector.tensor_copy(
                                dst[:, h, s * QTW:(s + 1) * QTW], ps[:])
                    # v for all 4 heads (kc-outer so wv streams once per slab)
                    vps = [psum.tile([P, DL], f32, tag="ps", name=f"vps{i}")
                           for i in range(4)]
                    for kc in range(KC):
                        wvt = wvpool.tile([P, DL], f32r, tag="wv")
                        nc.sync.dma_start(
                            wvt[:], wv.ap()[kc * P:(kc + 1) * P, :])
                        for tsub in range(4):
                            nc.tensor.matmul(
                                vps[tsub][:],
                                xTs[:, kc, tsub * P:(tsub + 1) * P],
                                wvt[:],
                                start=(kc == 0), stop=(kc == KC - 1))
                    for tsub in range(4):
                        nc.vector.tensor_copy(
                            v_sb[:, s * 4 + tsub, :], vps[tsub][:])

            # ------------ Phase B: causal attention ----------------------
            with (
                tc.tile_pool(name="b_e", bufs=6) as epool,
                tc.tile_pool(name="b_small", bufs=2) as bsmall,
                tc.tile_pool(name="b_out", bufs=3) as boutpool,
            ):
                for h in range(HL):
                    for qt in range(NQT):
                        nkb = (qt + 1) * 4
                        q_sl = slice(qt * QTW, (qt + 1) * QTW)
                        d_ps = psum.tile([1, QTW], f32, tag="ps", name="d_ps")
                        o_ps = psum.tile([P, QTW], f32, tag="ps", name="o_ps")
                        for kb in range(nkb):
                            s_ps = psum.tile([P, QTW], f32, tag="ps",
                                             name="s_ps")
                            nc.tensor.matmul(
                                s_ps[:],
                                kT_sb[:, h, kb * P:(kb + 1) * P],
                                qT_sb[:, h, q_sl],
                                start=True, stop=True)
                            e_sb = epool.tile([P, QTW], f32r, tag="e")
                            nc.scalar.activation(
                                e_sb[:], s_ps[:], AF.Exp, scale=float(SCALE))
                            if kb >= 4 * qt:
                                # diagonal block: keep q >= k, i.e.
                                # j >= i + m*128 with m = kb - 4*qt
                                m = kb - 4 * qt
                                nc.gpsimd.affine_select(
                                    out=e_sb[:], in_=e_sb[:],
                                    compare_op=mybir.AluOpType.is_ge,
                                    fill=0.0,
                                    base=-m * P,
                                    channel_multiplier=-1,
                                    pattern=[[1, QTW]])
                            nc.tensor.matmul(
                                d_ps[:], ones_col[:], e_sb[:],
                                start=(kb == 0), stop=(kb == nkb - 1))
                            nc.tensor.matmul(
                                o_ps[:], v_sb[:, kb, h * Hd:(h + 1) * Hd],
                                e_sb[:],
                                start=(kb == 0), stop=(kb == nkb - 1))
                        den = bsmall.tile([1, QTW], f32, tag="den")
                        nc.scalar.copy(den[:], d_ps[:])
                        rec = bsmall.tile([1, QTW], f32r, tag="rec")
                        with nc.allow_low_precision(
                                reason="f32r reciprocal, 2^-19 rel"):
                            nc.vector.reciprocal(rec[:], den[:])
                        bc_ps = psum.tile([P, QTW], f32, tag="ps",
                                          name="bc_ps")
                        nc.tensor.matmul(
                            bc_ps[:], ones_row[:], rec[:],
                            start=True, stop=True)
                        bc_sb = bsmall.tile([P, QTW], f32, tag="bc")
                        nc.scalar.copy(bc_sb[:], bc_ps[:])
                        at_sb = boutpool.tile([P, QTW], f32r, tag="at")
                        nc.vector.tensor_mul(at_sb[:], o_ps[:], bc_sb[:])
                        nc.sync.dma_start(atT_dram[h, :, q_sl], at_sb[:])

        # ------------ Phase C: output projection -------------------------
        with tc.tile_pool(name="c_pool", bufs=1) as cpool:
            wout_sb = cpool.tile([P, HL, D], f32r)
            nc.sync.dma_start(
                wout_sb[:], wout.ap().rearrange("(hl p) d -> p hl d", p=P))
            with (
                tc.tile_pool(name="c_at", bufs=8) as atpool,
                tc.tile_pool(name="c_y", bufs=2) as ypool,
            ):
                for tb in range(NTB):
                    t_sl = slice(tb * P, (tb + 1) * P)
                    ats = []
                    for h in range(HL):
                        at = atpool.tile([P, P], f32r, tag="at_in",
                                         name=f"at{h}")
                        nc.sync.dma_start(at[:], atT_dram[h, :, t_sl])
                        ats.append(at)
                    y_sb = ypool.tile([P, D], f32, tag="y")
                    for dc in range(D // QTW):
                        y_ps = psum.tile([P, QTW], f32, tag="ps", name="y_ps")
                        for h in range(HL):
                            nc.tensor.matmul(
                                y_ps[:],
                                ats[h][:],
                                wout_sb[:, h, dc * QTW:(dc + 1) * QTW],
                                start=(h == 0), stop=(h == HL - 1))
                        nc.vector.tensor_copy(
                            y_sb[:, dc * QTW:(dc + 1) * QTW], y_ps[:])
                    nc.sync.dma_start(y.ap()[t_sl, :], y_sb[:])


def prepare_in_maps(x, W_qkv, W_out):
    x = np.ascontiguousarray(np.asarray(x), dtype=np.float32)
    W_qkv = np.ascontiguousarray(np.asarray(W_qkv), dtype=np.float32)
    W_out = np.ascontiguousarray(np.asarray(W_out), dtype=np.float32)
    Wr = W_qkv.reshape(D, 3, H, Hd)
    Wo = W_out.reshape(H, Hd, D)
    xTs = [np.ascontiguousarray(x[b].T) for b in range(B)]
    in_maps = []
    for c in range(N_CORES):
        b, hg = c // 4, c % 4
        hs = slice(hg * HL, (hg + 1) * HL)
        in_maps.append({
            "xT": xTs[b],
            "wq": np.ascontiguousarray(Wr[:, 0, hs, :].reshape(D, DL)),
            "wk": np.ascontiguousarray(Wr[:, 1, hs, :].reshape(D, DL)),
            "wv": np.ascontiguousarray(Wr[:, 2, hs, :].reshape(D, DL)),
            "wout": np.ascontiguousarray(Wo[hs].reshape(DL, D)),
        })
    return in_maps


def combine_outputs(results):
    out = np.zeros((B, T, D), dtype=np.float32)
    for c in range(N_CORES):
        out[c // 4] += results[c]["y"]
    return out


_PROGRAM_CACHE = {}


def kernel(x, W_qkv, W_out):
    in_maps = prepare_in_maps(x, W_qkv, W_out)
    if 1 not in _PROGRAM_CACHE:
        _PROGRAM_CACHE[1] = build_program(1)
    nc = _PROGRAM_CACHE[1]
    res = run_bass_kernel_spmd(nc, in_maps, core_ids=list(range(N_CORES)))
    return combine_outputs(res.results)
